# revision 13
# baseline (speedup 1.0000x reference)
"""GCN layer on 8 trn2 cores.

Math: out = segment_sum((h@W * norm)[src], dst) * norm + bias
Linearity reorder: out = (segment_sum((h*norm)[src], dst) @ W) * norm + bias
=> aggregate input features first (partitioned by dst, no cross-core comm),
   GEMM + epilogue per dst shard afterwards.
"""
import numpy as np
from contextlib import ExitStack

import concourse.bass as bass
import concourse.bacc as bacc
import concourse.mybir as mybir
import concourse.tile as tile
from concourse.masks import make_identity
from concourse.bass_utils import run_bass_kernel_spmd

P = 128
N = 10000
D = 512
NCORES = 8
NPAD = 10240            # N padded to multiple of 128*NCORES
NPC = NPAD // NCORES    # nodes per core = 1280
NBLK = NPC // P         # dst blocks per core = 10
KC = D // P             # feature chunks = 4


def _build(C):
    """Build the single SPMD Bass program. C = edge chunks per dst block."""
    nc = bacc.Bacc(None, target_bir_lowering=False)
    f32 = mybir.dt.float32
    bf16 = mybir.dt.bfloat16
    i32 = mybir.dt.int32

    table = nc.declare_dram_parameter("table", [NPAD, D], bf16, isOutput=False)
    srci = nc.declare_dram_parameter("srci", [NBLK, P, C], i32, isOutput=False)
    rel = nc.declare_dram_parameter("rel", [NBLK, P, C], f32, isOutput=False)
    wt = nc.declare_dram_parameter("wt", [KC, P, D], f32, isOutput=False)
    nrm = nc.declare_dram_parameter("nrm", [NPC, 1], f32, isOutput=False)
    bi = nc.declare_dram_parameter("bi", [P, D], f32, isOutput=False)
    iota = nc.declare_dram_parameter("iota", [P, P], f32, isOutput=False)
    out = nc.declare_dram_parameter("out", [NPC, D], f32, isOutput=True)

    with tile.TileContext(nc) as tc, ExitStack() as ctx:
        const = ctx.enter_context(tc.tile_pool(name="const", bufs=1))
        epool = ctx.enter_context(tc.tile_pool(name="edges", bufs=NBLK))
        gpool = ctx.enter_context(tc.tile_pool(name="gath", bufs=8))
        spool = ctx.enter_context(tc.tile_pool(name="sel", bufs=8))
        apool = ctx.enter_context(tc.tile_pool(name="accs", bufs=NBLK))
        tpool = ctx.enter_context(tc.tile_pool(name="trs", bufs=4 * NBLK))
        opool = ctx.enter_context(tc.tile_pool(name="outs", bufs=NBLK))
        ps1 = ctx.enter_context(tc.tile_pool(name="ps1", bufs=2, space="PSUM"))
        pst = ctx.enter_context(tc.tile_pool(name="pst", bufs=4, space="PSUM"))
        ps2 = ctx.enter_context(tc.tile_pool(name="ps2", bufs=2, space="PSUM"))

        iota_t = const.tile([P, P], f32)
        nc.sync.dma_start(out=iota_t[:], in_=iota[:])
        ident_t = const.tile([P, P], f32)
        make_identity(nc, ident_t[:])
        bias_t = const.tile([P, D], f32)
        nc.sync.dma_start(out=bias_t[:], in_=bi[:])
        w_t = const.tile([P, KC * D], f32)
        for kc in range(KC):
            nc.sync.dma_start(out=w_t[:, kc * D:(kc + 1) * D], in_=wt[kc])

        for b in range(NBLK):
            idx_b = epool.tile([P, C], i32)
            nc.sync.dma_start(out=idx_b[:], in_=srci[b])
            rel_b = epool.tile([P, C], f32)
            nc.sync.dma_start(out=rel_b[:], in_=rel[b])
            nrm_b = epool.tile([P, 1], f32)
            nc.sync.dma_start(out=nrm_b[:], in_=nrm[b * P:(b + 1) * P, :])

            # accD[dst, feat] = segment-sum of gathered src rows for this
            # block, accumulated in PSUM across C edge chunks.
            accD = ps1.tile([P, D], f32, space="PSUM")
            for k in range(C):
                g_t = gpool.tile([P, D], bf16)
                nc.gpsimd.indirect_dma_start(
                    out=g_t[:],
                    out_offset=None,
                    in_=table[:],
                    in_offset=bass.IndirectOffsetOnAxis(ap=idx_b[:, k:k + 1], axis=0),
                )
                # S_T[e, j] = (rel[e] == j); padded edges have rel=-1 -> all 0
                s_t = spool.tile([P, P], bf16)
                nc.vector.tensor_tensor(
                    out=s_t[:],
                    in0=rel_b[:, k:k + 1].to_broadcast([P, P]),
                    in1=iota_t[:],
                    op=mybir.AluOpType.is_equal,
                )
                nc.tensor.matmul(
                    out=accD[:],
                    lhsT=s_t[:],
                    rhs=g_t[:],
                    start=(k == 0),
                    stop=(k == C - 1),
                )

            accS = apool.tile([P, D], f32)
            nc.vector.tensor_copy(out=accS[:], in_=accD[:])

            # out_ps[dst, :] = sum_kc A_kc @ W_kc (transpose chunks for lhsT)
            out_ps = ps2.tile([P, D], f32, space="PSUM")
            for kc in range(KC):
                tps = pst.tile([P, P], f32, space="PSUM")
                nc.tensor.transpose(
                    out=tps[:], in_=accS[:, kc * P:(kc + 1) * P],
                    identity=ident_t[:])
                lhsT_kc = tpool.tile([P, P], f32)
                nc.vector.tensor_copy(out=lhsT_kc[:], in_=tps[:])
                nc.tensor.matmul(
                    out=out_ps[:],
                    lhsT=lhsT_kc[:],
                    rhs=w_t[:, kc * D:(kc + 1) * D],
                    start=(kc == 0),
                    stop=(kc == KC - 1),
                )
            out_sb = opool.tile([P, D], f32)
            nc.vector.tensor_tensor(
                out=out_sb[:], in0=out_ps[:],
                in1=nrm_b[:].to_broadcast([P, D]),
                op=mybir.AluOpType.mult,
            )
            nc.vector.tensor_tensor(
                out=out_sb[:], in0=out_sb[:], in1=bias_t[:],
                op=mybir.AluOpType.add,
            )
            nc.sync.dma_start(out=out[b * P:(b + 1) * P, :], in_=out_sb[:])
    nc.compile()
    return nc


def _prep(h, norm, weight, bias, src, dst):
    import ml_dtypes
    hn = (h * norm).astype(np.float32)
    table = np.zeros((NPAD, D), dtype=ml_dtypes.bfloat16)
    table[:N] = hn.astype(ml_dtypes.bfloat16)

    src = np.asarray(src, dtype=np.int64)
    dst = np.asarray(dst, dtype=np.int64)
    core_of = dst // NPC
    blk_of = (dst % NPC) // P

    # chunk count: max edges landing in any (core, block), ceil to 128
    counts = np.zeros((NCORES, NBLK), dtype=np.int64)
    np.add.at(counts, (core_of, blk_of), 1)
    C = max(1, int(-(-counts.max() // P)))

    srci_all = np.zeros((NCORES, NBLK, P, C), dtype=np.int32)
    rel_all = np.full((NCORES, NBLK, P, C), -1.0, dtype=np.float32)
    gkey = core_of * NBLK + blk_of
    order = np.argsort(gkey, kind="stable")
    s_sorted = src[order]
    d_sorted = dst[order]
    g_sorted = gkey[order]
    starts = np.searchsorted(g_sorted, np.arange(NCORES * NBLK))
    ends = np.searchsorted(g_sorted, np.arange(NCORES * NBLK), side="right")
    for g in range(NCORES * NBLK):
        c, b = divmod(g, NBLK)
        lo, hi = starts[g], ends[g]
        cnt = hi - lo
        if cnt == 0:
            continue
        j = np.arange(cnt)
        srci_all[c, b, j % P, j // P] = s_sorted[lo:hi]
        rel_all[c, b, j % P, j // P] = (d_sorted[lo:hi] % P).astype(np.float32)

    normv = np.zeros((NPAD, 1), dtype=np.float32)
    normv[:N] = norm.astype(np.float32)
    wt = np.ascontiguousarray(weight.astype(np.float32).reshape(KC, P, D))
    bi = np.ascontiguousarray(
        np.broadcast_to(bias.astype(np.float32)[None, :], (P, D)))
    iota = np.ascontiguousarray(
        np.broadcast_to(np.arange(P, dtype=np.float32)[None, :], (P, P)))

    in_maps = []
    for c in range(NCORES):
        in_maps.append({
            "table": table,
            "srci": srci_all[c],
            "rel": rel_all[c],
            "wt": wt,
            "nrm": normv[c * NPC:(c + 1) * NPC],
            "bi": bi,
            "iota": iota,
        })
    return C, in_maps


def kernel(h, norm, weight, bias, src, dst):
    C, in_maps = _prep(h, norm, weight, bias, src, dst)
    nc = _build(C)
    res = run_bass_kernel_spmd(nc, in_maps, list(range(NCORES))).results
    out = np.concatenate(
        [np.asarray(res[c]["out"], dtype=np.float32) for c in range(NCORES)],
        axis=0)
    return out[:N]
